# revision 36
# baseline (speedup 1.0000x reference)
"""Trainium2 Bass kernel for nn_MergeMetaCNN (hypernetwork MLP -> grouped conv -> CNN).

Data-parallel over batch: 32 samples -> 8 NeuronCores, 4 samples each.

Per-core pipeline (all math on device):
  1. MLP (fp32 matmuls): hid = relu(W1^T fxT + b1); rawT = W2^T hid + b2,
     scaled per-row by 0.1/27 (filter part) / 0.1 (bias part).
  2. conv1 (grouped 3x3, per-sample dynamic filters) as ONE matmul pass:
     block-diagonal stationary [4*27, 4*8] (bf16), moving operand = im2col
     tile [108, rows, 226] DMA-gathered (single 4-dim-AP DMA per tile) from
     bf16 padded X planes staged host-side.
  3. conv2 (8->64, 3x3) per sample-pair: stationary [72, 64], moving =
     im2col [72, rows, 226] gathered from padded bf16 y planes; two samples
     share one [128, 1024] PSUM tile (sample A in partitions 0-63, sample B
     in 64-127) so each epilogue op covers 128 partitions.
  4. Epilogue relu(x + b) alternating ScalarE/VectorE, bf16 output staged in
     SBUF and stored to HBM as bf16 (host upcasts to fp32 after gather).
"""

import numpy as np
import ml_dtypes
from contextlib import ExitStack

import concourse.bass as bass
import concourse.tile as tile
from concourse import bacc, mybir
from concourse.bass_utils import run_bass_kernel_spmd

AP = bass.AP
f32 = mybir.dt.float32
bf16 = mybir.dt.bfloat16
AF = mybir.ActivationFunctionType
ALU = mybir.AluOpType

# Problem constants (hardcoded per contract)
B, CIN, H, W = 32, 3, 224, 224
TMP, K, FLAT, COUT = 8, 3, 128, 64
MLP_OUT = TMP * CIN * K * K + TMP  # 224
META = 0.1
NCORES = 8
SPC = B // NCORES                  # 4 samples per core
PH, PW = H + 2, W + 2              # 226 (zero-pad 1 on each side)
PLANE = PH * PW                    # 51076
PP = PLANE + 4                     # padded plane stride (tail slack for windows)
K27 = CIN * K * K                  # 27
K72 = TMP * K * K                  # 72
RT = 32                            # image rows per row-tile
NRT = H // RT                      # 7 row-tiles
RMM = 2                            # rows per matmul (PSUM bank: 448 fp32 <= 512)
NFREE = RMM * W                    # 448
NJP = RT // (2 * RMM)              # 8 psum tiles (2 matmuls each) per row-tile

_CACHE = {}


def build_module(repeat=1, loop_n=None):
    """Build + compile the single-core Bass module (SPMD across 8 cores).

    repeat>1 duplicates the conv pipeline instructions. loop_n wraps the
    pipeline in a hardware For_i loop executing it loop_n times with a
    constant instruction count -- wall-clock slope over loop_n isolates
    device execution time from NEFF load/dispatch overhead."""
    key = ("nc", repeat, loop_n)
    if key in _CACHE:
        return _CACHE[key]
    nc = bacc.Bacc("TRN2", target_bir_lowering=False, debug=False, num_devices=NCORES)

    # ---- DRAM I/O (per-core shapes) ----
    padX = nc.dram_tensor("padX", [SPC * CIN, PP], bf16, kind="ExternalInput")
    fxT = nc.dram_tensor("fxT", [FLAT, SPC], f32, kind="ExternalInput")
    W1 = nc.dram_tensor("W1", [FLAT, MLP_OUT], f32, kind="ExternalInput")
    b1 = nc.dram_tensor("b1", [MLP_OUT], f32, kind="ExternalInput")
    W2 = nc.dram_tensor("W2", [MLP_OUT + 1, MLP_OUT], f32, kind="ExternalInput")
    b2 = nc.dram_tensor("b2", [MLP_OUT], f32, kind="ExternalInput")
    cnn_wT = nc.dram_tensor("cnn_wT", [K72, COUT], f32, kind="ExternalInput")
    cnn_b = nc.dram_tensor("cnn_b", [COUT], f32, kind="ExternalInput")
    out = nc.dram_tensor("out", [SPC, COUT, H * W], bf16, kind="ExternalOutput")



    with tile.TileContext(nc) as tc, ExitStack() as ctx:
        cpool = ctx.enter_context(tc.tile_pool(name="consts", bufs=1))
        spool = ctx.enter_context(tc.tile_pool(name="stageA", bufs=1))
        mlp_ctx = ExitStack()
        mpsum = mlp_ctx.enter_context(tc.tile_pool(name="mlp_psum", bufs=2, space="PSUM"))

        # ================= Stage A: MLP + weight prep =================
        w1sb = cpool.tile([FLAT, MLP_OUT], f32)
        nc.sync.dma_start(w1sb[:], W1.ap())
        # W2e rows 0..223 = W2 (hid features), row 224 = b2 (ones-row trick)
        w2a = cpool.tile([128, MLP_OUT], f32)
        nc.sync.dma_start(w2a[:], W2.ap()[0:128, :])
        w2b = cpool.tile([97, MLP_OUT], f32)
        nc.sync.dma_start(w2b[:], W2.ap()[128:225, :])
        fx_sb = cpool.tile([FLAT, SPC], f32)
        nc.sync.dma_start(fx_sb[:], fxT.ap())
        b1a = cpool.tile([128, 1], f32)
        nc.sync.dma_start(b1a[:], b1.ap()[0:128].unsqueeze(1))
        b1b = cpool.tile([96, 1], f32)
        nc.sync.dma_start(b1b[:], b1.ap()[128:224].unsqueeze(1))
        # conv2 bias replicated for the [128 = 2 samples x 64] psum packing
        cnnb_sb = cpool.tile([128, 1], f32)
        nc.sync.dma_start(cnnb_sb[0:COUT, :], cnn_b.ap().unsqueeze(1))
        nc.sync.dma_start(cnnb_sb[COUT:2 * COUT, :], cnn_b.ap().unsqueeze(1))
        lhsT2 = cpool.tile([K72, COUT], bf16)
        nc.gpsimd.dma_start(lhsT2[:], cnn_wT.ap())  # cast f32 -> bf16

        WSCALE = META / K27

        # hid^T = relu(W1^T @ fxT + b1)   [224, SPC] in two partition chunks;
        # hidb row 96 = 1.0 (ones-row: picks up b2 from W2e's last row)
        ph_a = mpsum.tile([128, SPC], f32, tag="mp")
        nc.tensor.matmul(ph_a[:], lhsT=w1sb[:, 0:128], rhs=fx_sb[:], start=True, stop=True)
        hida = spool.tile([128, SPC], f32)
        nc.scalar.activation(hida[:], ph_a[:], func=AF.Relu, bias=b1a[:])
        ph_b = mpsum.tile([96, SPC], f32, tag="mp")
        nc.tensor.matmul(ph_b[:], lhsT=w1sb[:, 128:224], rhs=fx_sb[:], start=True, stop=True)
        hidb = spool.tile([97, SPC], f32)
        nc.vector.memset(hidb[96:97, :], 1.0)
        nc.scalar.activation(hidb[0:96, :], ph_b[:], func=AF.Relu, bias=b1b[:])

        # raw = (hid @ W2 + b2) * WSCALE, computed sample-major [SPC, 224]:
        # stationary = hid chunks, moving = W2e rows (uniform WSCALE; bias
        # rows corrected by x27 below)
        praw = mpsum.tile([SPC, MLP_OUT], f32, tag="mp")
        nc.tensor.matmul(praw[:], lhsT=hida[:], rhs=w2a[:], start=True, stop=False)
        nc.tensor.matmul(praw[:], lhsT=hidb[:], rhs=w2b[:], start=False, stop=True)
        raw_sb = spool.tile([SPC, MLP_OUT], bf16)
        nc.scalar.activation(raw_sb[:], praw[:], func=AF.Identity, scale=WSCALE)

        # conv1 stationary: block-diag [4*27, 4*8] bf16, partition order
        # (ky, s, ci, kx): lhsT1[ky*36 + s*9 + ci*3 + kx, s*8 + t] = wt[s][t,ci,ky,kx]
        # W2/b2 columns are host-permuted so rawT rows are already in
        # (ky, ci, kx, t) order -> one 3-dim-AP gather per sample.
        lhsT1 = cpool.tile([SPC * K27, SPC * TMP], bf16)
        nc.vector.memset(lhsT1[:], 0.0)
        for s in range(SPC):
            for ky in range(K):
                # dst partitions ky*36+s*9+(ci,kx), cols s*8+t <- raw_sb
                # partition s, cols ky*72 .. +72 (already (j, t)-ordered)
                nc.sync.dma_start(
                    lhsT1[ky * 36 + s * 9:ky * 36 + s * 9 + 9,
                          s * TMP:(s + 1) * TMP],
                    raw_sb[s:s + 1, ky * 72:(ky + 1) * 72],
                )
        # conv1 bias vector [32, 1]: bias1[s*8+t] = raw[s, 216+t]
        bias1h = cpool.tile([SPC * TMP, 1], bf16)
        nc.sync.dma_start(bias1h[:], raw_sb[:, 216:224].unsqueeze(2))
        # bias rows need scale 0.1, not 0.1/27 -> multiply by 27
        bias1 = cpool.tile([SPC * TMP, 1], f32)
        nc.vector.tensor_scalar_mul(bias1[:], bias1h[:], float(K27))

        mlp_ctx.close()  # release MLP PSUM banks for conv pools

        # ================= Stage B prep: padded bf16 y planes =================
        # padY lives in SBUF: planes (s,t) on 32 partitions, PP bf16 each.
        # conv1 epilogue writes the interior directly; zero the ring once.
        ypool_res = ctx.enter_context(tc.tile_pool(name="ypres", bufs=1))
        padY_sb = ypool_res.tile([SPC * TMP, PP], bf16)
        nc.vector.memset(padY_sb[:, 0:PW], 0.0)                    # top row
        nc.vector.memset(padY_sb[:, 225 * PW:PP], 0.0)             # bottom row + tail
        lr = padY_sb[:, PW:225 * PW].rearrange("p (r c) -> p r c", c=PW)
        nc.vector.memset(lr[:, :, 0:1], 0.0)                       # left col
        nc.vector.memset(lr[:, :, 225:226], 0.0)                   # right col

        # ================= Stage B: conv pipeline =================
        ic1 = ctx.enter_context(tc.tile_pool(name="ic1", bufs=2))
        ic2 = ctx.enter_context(tc.tile_pool(name="ic2", bufs=3))
        op_ = ctx.enter_context(tc.tile_pool(name="opool", bufs=2))
        ps1 = ctx.enter_context(tc.tile_pool(name="ps1", bufs=2, space="PSUM"))
        ps2 = ctx.enter_context(tc.tile_pool(name="ps2", bufs=3, space="PSUM"))

        ep_ctr = [0]

        def conv1_iter(r):
            r0 = r * RT
            t1 = ic1.tile([SPC * K27, RT, PW], bf16, name=f"t1_{ep_ctr[0]}_{r}", tag="t1")
            # partition (ky, s, ci, kx) <- padX plane (s,ci), shifted ky*PW+kx
            # single 4-dim-AP DMA: dims (ky, plane, kx, flat-rows)
            for ky in range(K):
                src = AP(
                    tensor=padX,
                    offset=(r0 + ky) * PW,
                    ap=[[PP, SPC * CIN], [1, K], [1, RT * PW]],
                )
                # SWDGE ring: keeps t1 loads off the SP ring so they don't
                # queue behind t2 loads waiting on conv1 epilogues
                nc.gpsimd.dma_start(t1[ky * 36:(ky + 1) * 36], src)
            for jp in range(2 * NJP):
                # single-bank psum: one 448-wide matmul (2 rows) + 1 epilogue
                p1 = ps1.tile([SPC * TMP, 512], f32,
                              name=f"p1_{ep_ctr[0]}_{r}_{jp}", tag="p1")
                nc.tensor.matmul(
                    p1[:, 0:NFREE], lhsT=lhsT1[:],
                    rhs=t1[:, 2 * jp:2 * jp + 2, 0:W], start=True, stop=True,
                )
                # write y rows (r0+2jp, +1) straight into padY_sb interior
                dst = AP(
                    tensor=padY_sb.tensor,
                    offset=(1 + r0 + jp * 2) * PW + 1,
                    ap=[[PP, SPC * TMP], [PW, 2], [1, W]],
                )
                pv = AP(
                    tensor=p1.tensor, offset=0,
                    ap=[[512, SPC * TMP], [W, 2], [1, W]],
                )
                if jp % 2 == 0:
                    nc.scalar.activation(dst, pv, func=AF.Identity, bias=bias1[:])
                else:
                    nc.vector.tensor_scalar(
                        dst, pv, bias1[:], 0.0, op0=ALU.add, op1=ALU.bypass
                    )

        pending_store = []

        def flush_store():
            while pending_store:
                dst, osb = pending_store.pop(0)
                nc.scalar.dma_start(dst, osb[:])

        HR = RT // 2  # 16 rows per half-tile

        def conv2_load_half(s0, r, h):
            """Load im2col half-tiles (16 rows) for samples (s0, s0+1)."""
            r0 = r * RT + h * HR
            t2s = []
            for si in range(2):
                s = s0 + si
                t2 = ic2.tile([K72, HR, PW], bf16,
                              name=f"t2_{s}_{r}_{h}_{ld_ctr[0]}", tag="t2")
                # partition (dy, t, dx) <- padY_sb plane (s,t), shifted dy*PW+dx
                for dy in range(K):
                    src = AP(
                        tensor=padY_sb.tensor,
                        offset=s * TMP * PP + (r0 + dy) * PW,
                        ap=[[PP, TMP], [1, K], [1, HR * PW]],
                    )
                    nc.sync.dma_start(t2[dy * 24:(dy + 1) * 24], src)
                t2s.append(t2)
            ld_ctr[0] += 1
            return t2s

        def conv2_compute_half(s0, r, h, t2s):
            """conv2 for samples (s0, s0+1), half-tile (r, h): both samples
            share each [128, 1024] psum tile (A in 0:64, B in 64:128)."""
            if h == 0:
                osb = op_.tile([2 * COUT, RT * W], bf16,
                               name=f"o_{s0}_{r}", tag="o")
                cur_osb[0] = osb
                flush_store()
            else:
                osb = cur_osb[0]
            for jp in range(NJP // 2):
                p2 = ps2.tile([2 * COUT, 1024], f32,
                              name=f"p2_{s0}_{r}_{h}_{jp}", tag="p2")
                for si in range(2):
                    nc.tensor.matmul(
                        p2[si * COUT:(si + 1) * COUT, 0:NFREE], lhsT=lhsT2[:],
                        rhs=t2s[si][:, 4 * jp:4 * jp + 2, 0:W],
                        start=True, stop=True,
                    )
                    nc.tensor.matmul(
                        p2[si * COUT:(si + 1) * COUT, 512:512 + NFREE],
                        lhsT=lhsT2[:],
                        rhs=t2s[si][:, 4 * jp + 2:4 * jp + 4, 0:W],
                        start=True, stop=True,
                    )
                # one 128-partition epilogue op per psum tile: relu(x + b)
                pv = p2.rearrange("p (a b) -> p a b", a=2)[:, :, 0:NFREE]
                obase = (h * NJP // 2 + jp) * 2 * NFREE
                oslice = osb[:, obase:obase + 2 * NFREE].rearrange(
                    "p (a b) -> p a b", a=2)
                if ep_ctr[0] % 2 == 0:
                    nc.scalar.activation(oslice, pv, func=AF.Relu,
                                         bias=cnnb_sb[:])
                else:
                    nc.vector.tensor_scalar(
                        oslice, pv, cnnb_sb[:], 0.0, op0=ALU.add, op1=ALU.max
                    )
                ep_ctr[0] += 1
            if h == 1:
                # bf16 store via HWDGE on the ACT ring. Deferred: issued at
                # the NEXT pair so its waits never stall the ACT ring.
                dst = AP(
                    tensor=out,
                    offset=s0 * COUT * H * W + r * RT * W,
                    ap=[[COUT * H * W, 2], [H * W, COUT], [1, RT * W]],
                )
                pending_store.append((dst, osb))

        ld_ctr = [0]
        cur_osb = [None]

        def pipeline():
            conv1_iter(0)
            conv1_iter(1)
            seq = [(s0, r, h) for r in range(NRT) for s0 in (0, 2)
                   for h in (0, 1)]
            pending = conv2_load_half(*seq[0])
            for i, (s0, r, h) in enumerate(seq):
                if s0 == 0 and h == 0 and r + 2 < NRT:
                    conv1_iter(r + 2)
                t2s = pending
                if i + 1 < len(seq):
                    pending = conv2_load_half(*seq[i + 1])
                conv2_compute_half(s0, r, h, t2s)
            flush_store()

        if loop_n is not None:
            hints = [mybir.EngineType.PE, mybir.EngineType.Activation,
                     mybir.EngineType.DVE, mybir.EngineType.SP,
                     mybir.EngineType.Pool]
            with tc.For_i(0, loop_n, 1, hint_engines=hints):
                pipeline()
        else:
            for _rep in range(repeat):
                pipeline()

    nc.compile()
    _CACHE[key] = nc
    return nc


def make_in_maps(X, flat_x, W1, b1, W2, b2, cnn_w, cnn_b):
    X = np.asarray(X, np.float32)
    flat_x = np.asarray(flat_x, np.float32)
    W1 = np.asarray(W1, np.float32)
    b1 = np.asarray(b1, np.float32)
    W2 = np.asarray(W2, np.float32)
    b2 = np.asarray(b2, np.float32)
    cnn_w = np.asarray(cnn_w, np.float32)
    cnn_b = np.asarray(cnn_b, np.float32)

    img = np.zeros((B, CIN, PH, PW), np.float32)
    img[:, :, 1:1 + H, 1:1 + W] = X
    Xp = np.zeros((B, CIN, PP), np.float32)
    Xp[:, :, :PLANE] = img.reshape(B, CIN, PLANE)
    Xp = Xp.astype(ml_dtypes.bfloat16)

    # Permute W2/b2 columns so rawT's filter rows come out in
    # (ky, ci, kx, t) order: raw'[ky*72+ci*24+kx*8+t] = raw[t*27+ci*9+ky*3+kx]
    perm = np.empty(MLP_OUT, np.int64)
    for ky in range(K):
        for ci in range(CIN):
            for kx in range(K):
                for t in range(TMP):
                    perm[ky * 72 + ci * 24 + kx * 8 + t] = (
                        t * K27 + ci * 9 + ky * 3 + kx)
    perm[TMP * K27:] = np.arange(TMP * K27, MLP_OUT)
    # stack permuted b2 as the last row of W2 (ones-row bias trick)
    W2 = np.ascontiguousarray(
        np.vstack([W2[:, perm], b2[perm][None, :]]))
    fxT_full = np.ascontiguousarray(flat_x.T)                  # [128, 32]
    cnn_wT = np.ascontiguousarray(
        cnn_w.transpose(2, 1, 3, 0).reshape(K72, COUT))        # [72,64] (dy,t,dx,co)

    in_maps = []
    for i in range(NCORES):
        sl = slice(i * SPC, (i + 1) * SPC)
        in_maps.append({
            "padX": np.ascontiguousarray(Xp[sl].reshape(SPC * CIN, PP)),
            "fxT": np.ascontiguousarray(fxT_full[:, sl]),
            "W1": W1, "b1": b1, "W2": W2, "b2": b2,
            "cnn_wT": cnn_wT, "cnn_b": cnn_b,
        })
    return in_maps


def kernel(X, flat_x, W1, b1, W2, b2, cnn_w, cnn_b):
    nc = build_module()
    in_maps = make_in_maps(X, flat_x, W1, b1, W2, b2, cnn_w, cnn_b)
    res = run_bass_kernel_spmd(nc, in_maps, core_ids=list(range(NCORES)))
    outs = [
        np.asarray(res.results[i]["out"]).astype(np.float32).reshape(
            SPC, COUT, H, W)
        for i in range(NCORES)
    ]
    return np.concatenate(outs, axis=0)


# revision 44
# speedup vs baseline: 2.1814x; 2.1814x over previous
"""Trainium2 Bass kernel for nn_MergeMetaCNN (hypernetwork MLP -> grouped conv -> CNN).

Data-parallel over batch: 32 samples -> 8 NeuronCores, 4 samples each.

Per-core pipeline (all math on device):
  1. MLP: layer 1 feature-major (stationary W1), layer 2 sample-major
     (stationary hid, moving W2e with b2 folded in via a ones-row) so the
     per-sample conv filters land in SBUF already sample-major.
  2. conv1 (grouped 3x3, per-sample dynamic filters): one matmul pass with a
     block-diagonal stationary [108, 96] whose columns are (dx, s, t) -- the
     weights replicated 3x over dx. Each psum bank gets 3 epilogue ops (one
     per dx block) writing dx-SHIFTED padded-y copies into padYr [96, PP].
  3. conv2 (8->64, 3x3) reads its moving operand DIRECTLY from padYr (no
     im2col DMA at all): per psum bank, 3 accumulating matmuls (one per dy,
     row-shifted views) with block-diag stationaries [96, 128] covering two
     samples (A in psum partitions 0:64, B in 64:128).
  4. Epilogue relu(x + b) alternating ScalarE/VectorE, bf16 output staged in
     SBUF, stored to HBM as bf16 (host upcasts to fp32 after gather).
"""

import numpy as np
import ml_dtypes
from contextlib import ExitStack

import concourse.bass as bass
import concourse.tile as tile
from concourse import bacc, mybir
from concourse.bass_utils import run_bass_kernel_spmd

AP = bass.AP
f32 = mybir.dt.float32
bf16 = mybir.dt.bfloat16
AF = mybir.ActivationFunctionType
ALU = mybir.AluOpType

# Problem constants (hardcoded per contract)
B, CIN, H, W = 32, 3, 224, 224
TMP, K, FLAT, COUT = 8, 3, 128, 64
MLP_OUT = TMP * CIN * K * K + TMP  # 224
META = 0.1
NCORES = 8
SPC = B // NCORES                  # 4 samples per core
PH, PW = H + 2, W + 2              # 226 (zero-pad 1 on each side)
PLANE = PH * PW                    # 51076
PP = PLANE + 4                     # padded plane stride (tail slack for windows)
K27 = CIN * K * K                  # 27
K72 = TMP * K * K                  # 72
RT = 32                            # image rows per row-tile
NRT = H // RT                      # 7 row-tiles
RMM = 2                            # rows per matmul (PSUM bank: 448 fp32 <= 512)
NFREE = RMM * W                    # 448
NJP = RT // (2 * RMM)              # 8 two-bank psum tiles per (pair, row-tile)
NPL = 96                           # padYr planes: (dx, s, t)

_CACHE = {}


def build_module(repeat=1, loop_n=None, variant="full"):
    """Build + compile the single-core Bass module (SPMD across 8 cores)."""
    key = ("nc", repeat, loop_n, variant)
    if key in _CACHE:
        return _CACHE[key]
    nc = bacc.Bacc("TRN2", target_bir_lowering=False, debug=False, num_devices=NCORES)

    # ---- DRAM I/O (per-core shapes) ----
    padX = nc.dram_tensor("padX", [SPC * CIN, PP], bf16, kind="ExternalInput")
    fxT = nc.dram_tensor("fxT", [FLAT, SPC], f32, kind="ExternalInput")
    W1 = nc.dram_tensor("W1", [FLAT, MLP_OUT], f32, kind="ExternalInput")
    b1 = nc.dram_tensor("b1", [MLP_OUT], f32, kind="ExternalInput")
    W2 = nc.dram_tensor("W2", [MLP_OUT + 1, MLP_OUT], f32, kind="ExternalInput")
    cnn_w6 = nc.dram_tensor("cnn_w6", [2 * K, NPL, 2 * COUT], bf16,
                            kind="ExternalInput")
    cnn_b = nc.dram_tensor("cnn_b", [COUT], f32, kind="ExternalInput")
    out = nc.dram_tensor("out", [SPC, COUT, H * W], bf16, kind="ExternalOutput")

    with tile.TileContext(nc) as tc, ExitStack() as ctx:
        cpool = ctx.enter_context(tc.tile_pool(name="consts", bufs=1))
        spool = ctx.enter_context(tc.tile_pool(name="stageA", bufs=1))
        mlp_ctx = ExitStack()
        mpsum = mlp_ctx.enter_context(tc.tile_pool(name="mlp_psum", bufs=2, space="PSUM"))

        # ================= Stage A: MLP + weight prep =================
        w1sb = cpool.tile([FLAT, MLP_OUT], f32)
        nc.sync.dma_start(w1sb[:], W1.ap())
        # W2e rows 0..223 = W2 (hid features), row 224 = b2 (ones-row trick)
        w2a = cpool.tile([128, MLP_OUT], f32)
        nc.sync.dma_start(w2a[:], W2.ap()[0:128, :])
        w2b = cpool.tile([97, MLP_OUT], f32)
        nc.sync.dma_start(w2b[:], W2.ap()[128:225, :])
        fx_sb = cpool.tile([FLAT, SPC], f32)
        nc.sync.dma_start(fx_sb[:], fxT.ap())
        b1a = cpool.tile([128, 1], f32)
        nc.sync.dma_start(b1a[:], b1.ap()[0:128].unsqueeze(1))
        b1b = cpool.tile([96, 1], f32)
        nc.sync.dma_start(b1b[:], b1.ap()[128:224].unsqueeze(1))
        # conv2 bias replicated for the [128 = 2 samples x 64] psum packing
        cnnb_sb = cpool.tile([2 * COUT, 1], f32)
        nc.sync.dma_start(cnnb_sb[0:COUT, :], cnn_b.ap().unsqueeze(1))
        nc.sync.dma_start(cnnb_sb[COUT:2 * COUT, :], cnn_b.ap().unsqueeze(1))
        # conv2 stationaries: [96, 128] per (pair, dy), block-diag over samples
        lhsT2 = []
        for i in range(2 * K):
            t = cpool.tile([NPL, 2 * COUT], bf16, name=f"lhsT2_{i}",
                           tag=f"lhsT2_{i}")
            nc.sync.dma_start(t[:], cnn_w6.ap()[i])
            lhsT2.append(t)

        WSCALE = META / K27

        # hid^T = relu(W1^T @ fxT + b1)   [224, SPC] in two partition chunks;
        # hidb row 96 = 1.0 (ones-row: picks up b2 from W2e's last row)
        ph_a = mpsum.tile([128, SPC], f32, tag="mp")
        nc.tensor.matmul(ph_a[:], lhsT=w1sb[:, 0:128], rhs=fx_sb[:], start=True, stop=True)
        hida = spool.tile([128, SPC], f32)
        nc.scalar.activation(hida[:], ph_a[:], func=AF.Relu, bias=b1a[:])
        ph_b = mpsum.tile([96, SPC], f32, tag="mp")
        nc.tensor.matmul(ph_b[:], lhsT=w1sb[:, 128:224], rhs=fx_sb[:], start=True, stop=True)
        hidb = spool.tile([97, SPC], f32)
        nc.vector.memset(hidb[96:97, :], 1.0)
        nc.scalar.activation(hidb[0:96, :], ph_b[:], func=AF.Relu, bias=b1b[:])

        # raw = (hid @ W2 + b2) * WSCALE, computed sample-major [SPC, 224]
        praw = mpsum.tile([SPC, MLP_OUT], f32, tag="mp")
        nc.tensor.matmul(praw[:], lhsT=hida[:], rhs=w2a[:], start=True, stop=False)
        nc.tensor.matmul(praw[:], lhsT=hidb[:], rhs=w2b[:], start=False, stop=True)
        raw_sb = spool.tile([SPC, MLP_OUT], bf16)
        nc.scalar.activation(raw_sb[:], praw[:], func=AF.Identity, scale=WSCALE)

        # conv1 stationary: block-diag [108, 96] bf16; rows (ky, s, ci, kx),
        # cols (dx, s, t) with the same weights in each 32-col dx block.
        # W2's columns are host-permuted so raw rows are (ky, ci, kx, t).
        lhsT1 = cpool.tile([SPC * K27, NPL], bf16)
        nc.vector.memset(lhsT1[:], 0.0)
        for s in range(SPC):
            for ky in range(K):
                nc.sync.dma_start(
                    lhsT1[ky * 36 + s * 9:ky * 36 + s * 9 + 9,
                          s * TMP:(s + 1) * TMP],
                    raw_sb[s:s + 1, ky * 72:(ky + 1) * 72],
                )
        # replicate cols 0:32 -> 32:64, 64:96 (same filters per dx block)
        nc.scalar.copy(lhsT1[:, 32:64], lhsT1[:, 0:32])
        nc.scalar.copy(lhsT1[:, 64:96], lhsT1[:, 0:32])

        # conv1 bias vector [32, 1]: bias1[s*8+t] = raw[s, 216+t]
        bias1h = cpool.tile([SPC * TMP, 1], bf16)
        nc.sync.dma_start(bias1h[:], raw_sb[:, 216:224].unsqueeze(2))
        # bias rows need scale 0.1, not 0.1/27 -> multiply by 27
        bias1 = cpool.tile([SPC * TMP, 1], f32)
        nc.vector.tensor_scalar_mul(bias1[:], bias1h[:], float(K27))

        mlp_ctx.close()  # release MLP PSUM banks for conv pools

        # ================= Stage B prep: padded bf16 y planes =================
        # padYr [96, PP]: plane (dx, s, t) holds P[s,t] column-shifted so
        # conv2's rhs free offset is uniform: padYr[dx*32+p, r, c] =
        # P[p, r, c+dx-1]. conv1 epilogue writes interiors; zero the ring once.
        ypool_res = ctx.enter_context(tc.tile_pool(name="ypres", bufs=1))
        padY_sb = ypool_res.tile([NPL, PP], bf16)
        nc.vector.memset(padY_sb[:, 0:PW], 0.0)                    # top row
        nc.vector.memset(padY_sb[:, 225 * PW:PP], 0.0)             # bottom row + tail
        lr = padY_sb[:, PW:225 * PW].rearrange("p (r c) -> p r c", c=PW)
        nc.vector.memset(lr[:, :, 0:2], 0.0)                       # left cols
        nc.vector.memset(lr[:, :, 224:226], 0.0)                   # right cols

        # ================= Stage B: conv pipeline =================
        ic1 = ctx.enter_context(tc.tile_pool(name="ic1", bufs=2))
        op_ = ctx.enter_context(tc.tile_pool(name="opool", bufs=3))
        ps1 = ctx.enter_context(tc.tile_pool(name="ps1", bufs=2, space="PSUM"))
        ps2 = ctx.enter_context(tc.tile_pool(name="ps2", bufs=3, space="PSUM"))

        ep_ctr = [0]

        def conv1_iter(r):
            r0 = r * RT
            t1 = ic1.tile([SPC * K27, RT, PW], bf16, name=f"t1_{r}", tag="t1")
            # partition (ky, s, ci, kx) <- padX plane (s,ci), shifted ky*PW+kx
            for ky in range(K):
                src = AP(
                    tensor=padX,
                    offset=(r0 + ky) * PW,
                    ap=[[PP, SPC * CIN], [1, K], [1, RT * PW]],
                )
                nc.gpsimd.dma_start(t1[ky * 36:(ky + 1) * 36], src)
            for jp in range(2 * NJP):
                # single-bank psum: one 448-wide matmul (2 rows), 3 epilogues
                p1 = ps1.tile([NPL, 512], f32, name=f"p1_{r}_{jp}", tag="p1")
                nc.tensor.matmul(
                    p1[:, 0:NFREE], lhsT=lhsT1[:],
                    rhs=t1[:, 2 * jp:2 * jp + 2, 0:W], start=True, stop=True,
                )
                # per-dx epilogue: y rows (r0+2jp, +1) -> padYr block dx, with
                # column shift (2 - dx)
                for dx in range(K):
                    dst = AP(
                        tensor=padY_sb.tensor,
                        offset=(dx * 32) * PP + (1 + r0 + jp * 2) * PW + (2 - dx),
                        ap=[[PP, SPC * TMP], [PW, 2], [1, W]],
                    )
                    pv = AP(
                        tensor=p1.tensor, offset=dx * 32 * 512,
                        ap=[[512, SPC * TMP], [W, 2], [1, W]],
                    )
                    eng = (jp + dx) % 2
                    if eng == 0:
                        nc.scalar.activation(dst, pv, func=AF.Identity,
                                             bias=bias1[:])
                    else:
                        nc.vector.tensor_scalar(
                            dst, pv, bias1[:], 0.0, op0=ALU.add, op1=ALU.bypass
                        )

        pending_store = []

        def flush_store():
            while pending_store:
                dst, osb = pending_store.pop(0)
                if variant != "nostore":
                    nc.scalar.dma_start(dst, osb[:])

        def conv2_pair(pi, r):
            """conv2 for samples (2pi, 2pi+1), row-tile r: rhs read directly
            from padYr; psum [128, 1024] = 2 banks x (2 samples x 64 couts)."""
            r0 = r * RT
            osb = op_.tile([2 * COUT, RT * W], bf16, name=f"o_{pi}_{r}", tag="o")
            flush_store()
            for jp in range(NJP):
                p2 = ps2.tile([2 * COUT, 1024], f32,
                              name=f"p2_{pi}_{r}_{jp}", tag="p2")
                for half in range(2):
                    R0 = r0 + 4 * jp + 2 * half
                    for dy in range(K):
                        rhs = AP(
                            tensor=padY_sb.tensor,
                            offset=(R0 + dy) * PW + 1,
                            ap=[[PP, NPL], [PW, 2], [1, W]],
                        )
                        nc.tensor.matmul(
                            p2[:, half * 512:half * 512 + NFREE],
                            lhsT=lhsT2[pi * K + dy][:], rhs=rhs,
                            start=(dy == 0), stop=(dy == 2),
                        )
                # one 128-partition epilogue op per psum tile: relu(x + b)
                pv = p2.rearrange("p (a b) -> p a b", a=2)[:, :, 0:NFREE]
                obase = jp * 2 * NFREE
                oslice = osb[:, obase:obase + 2 * NFREE].rearrange(
                    "p (a b) -> p a b", a=2)
                if ep_ctr[0] % 2 == 0:
                    nc.scalar.activation(oslice, pv, func=AF.Relu,
                                         bias=cnnb_sb[:])
                else:
                    nc.vector.tensor_scalar(
                        oslice, pv, cnnb_sb[:], 0.0, op0=ALU.add, op1=ALU.max
                    )
                ep_ctr[0] += 1
            # bf16 store via HWDGE on the ACT ring; dst partitions (sample,
            # cout). Deferred: issued at the NEXT pair call so its waits are
            # already satisfied and never stall the ACT ring.
            dst = AP(
                tensor=out,
                offset=2 * pi * COUT * H * W + r0 * W,
                ap=[[COUT * H * W, 2], [H * W, COUT], [1, RT * W]],
            )
            pending_store.append((dst, osb))

        def pipeline():
            conv1_iter(0)
            conv1_iter(1)
            for r in range(NRT):
                if r + 2 < NRT:
                    conv1_iter(r + 2)
                for pi in (0, 1):
                    conv2_pair(pi, r)
            flush_store()

        if loop_n is not None:
            hints = [mybir.EngineType.PE, mybir.EngineType.Activation,
                     mybir.EngineType.DVE, mybir.EngineType.SP,
                     mybir.EngineType.Pool]
            with tc.For_i(0, loop_n, 1, hint_engines=hints):
                pipeline()
        else:
            for _rep in range(repeat):
                pipeline()

    nc.compile()
    _CACHE[key] = nc
    return nc


def make_in_maps(X, flat_x, W1, b1, W2, b2, cnn_w, cnn_b):
    X = np.asarray(X, np.float32)
    flat_x = np.asarray(flat_x, np.float32)
    W1 = np.asarray(W1, np.float32)
    b1 = np.asarray(b1, np.float32)
    W2 = np.asarray(W2, np.float32)
    b2 = np.asarray(b2, np.float32)
    cnn_w = np.asarray(cnn_w, np.float32)
    cnn_b = np.asarray(cnn_b, np.float32)

    img = np.zeros((B, CIN, PH, PW), np.float32)
    img[:, :, 1:1 + H, 1:1 + W] = X
    Xp = np.zeros((B, CIN, PP), np.float32)
    Xp[:, :, :PLANE] = img.reshape(B, CIN, PLANE)
    Xp = Xp.astype(ml_dtypes.bfloat16)

    # Permute W2/b2 columns so raw's filter block comes out in
    # (ky, ci, kx, t) order: raw'[ky*72+ci*24+kx*8+t] = raw[t*27+ci*9+ky*3+kx]
    perm = np.empty(MLP_OUT, np.int64)
    for ky in range(K):
        for ci in range(CIN):
            for kx in range(K):
                for t in range(TMP):
                    perm[ky * 72 + ci * 24 + kx * 8 + t] = (
                        t * K27 + ci * 9 + ky * 3 + kx)
    perm[TMP * K27:] = np.arange(TMP * K27, MLP_OUT)
    # stack permuted b2 as the last row of W2 (ones-row bias trick)
    W2 = np.ascontiguousarray(
        np.vstack([W2[:, perm], b2[perm][None, :]]))

    fxT_full = np.ascontiguousarray(flat_x.T)                  # [128, 32]

    # conv2 stationaries [2 pairs * 3 dy, 96, 128]: rows (dx, s, t) block-diag
    # over the pair's two samples in the columns (sc, co)
    w6 = np.zeros((2 * K, NPL, 2 * COUT), np.float32)
    for pi in range(2):
        for dy in range(K):
            for dx in range(K):
                for sc in range(2):
                    s = 2 * pi + sc
                    # rows dx*32 + s*8 + t, cols sc*64 + co
                    w6[pi * K + dy,
                       dx * 32 + s * TMP:dx * 32 + s * TMP + TMP,
                       sc * COUT:(sc + 1) * COUT] = cnn_w[:, :, dy, dx].T
    w6 = w6.astype(ml_dtypes.bfloat16)

    in_maps = []
    for i in range(NCORES):
        sl = slice(i * SPC, (i + 1) * SPC)
        in_maps.append({
            "padX": np.ascontiguousarray(Xp[sl].reshape(SPC * CIN, PP)),
            "fxT": np.ascontiguousarray(fxT_full[:, sl]),
            "W1": W1, "b1": b1, "W2": W2,
            "cnn_w6": w6, "cnn_b": cnn_b,
        })
    return in_maps


def kernel(X, flat_x, W1, b1, W2, b2, cnn_w, cnn_b):
    nc = build_module()
    in_maps = make_in_maps(X, flat_x, W1, b1, W2, b2, cnn_w, cnn_b)
    res = run_bass_kernel_spmd(nc, in_maps, core_ids=list(range(NCORES)))
    outs = [
        np.asarray(res.results[i]["out"]).astype(np.float32).reshape(
            SPC, COUT, H, W)
        for i in range(NCORES)
    ]
    return np.concatenate(outs, axis=0)
